# revision 2
# baseline (speedup 1.0000x reference)
"""GRU-with-reset Trainium2 kernel, 8-way tensor-parallel over hidden dim.

Terminations split the sequence into independent segments (h resets to 0).
Segments are sorted by length desc; round k batches the k-th step of every
still-alive segment (a prefix of the sorted order). Each round is one
batched matmul vs the full hidden state + gate elementwise + an 8-core
AllGather of the new h slices. All matmuls bf16 (fp32 PSUM accumulate),
elementwise fp32.
"""

import os
import sys
import types

import numpy as np
import ml_dtypes

BF = ml_dtypes.bfloat16
D = 2048
NC = 8
SLICE = D // NC          # 256 hidden per core
MT = 6                   # m-tiles per core: r0 r1 z0 z1 n0 n1
KT = D // 128            # 16 k-tiles


def _maybe_enable_trace():
    """Register the axon NTFF hook so trace=True works (test/profiling only)."""
    if not os.environ.get("BASS_TRACE"):
        return False
    try:
        mod = types.ModuleType("antenv.axon_hooks")
        mod._hook = None
        mod.set_axon_ntff_profile_hook = lambda h: setattr(mod, "_hook", h)
        mod.get_axon_ntff_profile_hook = lambda: mod._hook
        sys.modules["antenv.axon_hooks"] = mod
        import antenv

        antenv.axon_hooks = mod
        from trn_agent_boot.trn_boot import _ntff_profile_via_ctypes

        h = _ntff_profile_via_ctypes("/opt/axon/libaxon_pjrt.so")
        if h is not None:
            mod.set_axon_ntff_profile_hook(h)
            return True
    except Exception:
        pass
    return False


def _segments(term):
    T = len(term)
    starts = [0] + [t for t in range(1, T) if term[t]]
    lens = [
        (starts[i + 1] if i + 1 < len(starts) else T) - starts[i]
        for i in range(len(starts))
    ]
    order = sorted(range(len(starts)), key=lambda i: -lens[i])
    p = [starts[i] for i in order]
    l = [lens[i] for i in order]
    Lmax = l[0]
    Bk = [sum(1 for x in l if x > k) for k in range(Lmax)]
    rows = []
    for k in range(Lmax):
        for j in range(Bk[k]):
            rows.append(p[j] + k)
    return p, l, Bk, np.array(rows)


def _chunks(B):
    nb = (B + 511) // 512
    base, rem = divmod(B, nb)
    out = []
    for i in range(nb):
        out.append(base + (1 if i < rem else 0))
    return out


def _build(Bk):
    import concourse.bacc as bacc
    import concourse.tile as tile
    import concourse.mybir as mybir

    f32 = mybir.dt.float32
    bf16 = mybir.dt.bfloat16
    nc = bacc.Bacc("TRN2", target_bir_lowering=False, debug=False, num_devices=NC)
    T = 1024
    B0 = Bk[0]
    R = len(Bk)

    xT_d = nc.dram_tensor("xT", [D, T], bf16, kind="ExternalInput")
    wi_d = nc.dram_tensor("wi", [D, 768], bf16, kind="ExternalInput")
    wh_d = nc.dram_tensor("wh", [D, 768], bf16, kind="ExternalInput")
    bi_d = nc.dram_tensor("bi", [MT, 128], f32, kind="ExternalInput")
    bhn_d = nc.dram_tensor("bhn", [1, SLICE], bf16, kind="ExternalInput")
    h0_d = nc.dram_tensor("h0", [D, B0], bf16, kind="ExternalInput")
    h0own_d = nc.dram_tensor("h0own", [SLICE, B0], bf16, kind="ExternalInput")
    eye_d = nc.dram_tensor("eye", [128, 128], bf16, kind="ExternalInput")
    y_d = nc.dram_tensor("y", [SLICE, T], bf16, kind="ExternalOutput")

    with tile.TileContext(nc) as tc:
        with (
            tc.tile_pool(name="sb", bufs=1) as sb,
            tc.tile_pool(name="work", bufs=2) as wk,
            tc.tile_pool(name="psum", bufs=1, space="PSUM") as pp,
        ):
            wh_sb = sb.tile([128, KT * 768], bf16, tag="wh")
            wi_sb = sb.tile([128, KT * 768], bf16, tag="wi")
            xT_sb = sb.tile([128, KT * T], bf16, tag="xT")
            gx_sb = sb.tile([128, MT * T], bf16, tag="gx")
            H = sb.tile([128, KT * B0], bf16, tag="H")
            bi_sb = sb.tile([128, MT], f32, tag="bi")
            bhn_sb = sb.tile([1, SLICE], bf16, tag="bhn")
            eye_sb = sb.tile([128, 128], bf16, tag="eye")
            ones_sb = sb.tile([1, B0], bf16, tag="ones")

            nc.sync.dma_start(eye_sb[:, :], eye_d[:, :])
            nc.vector.memset(ones_sb[:, :], 1.0)
            nc.sync.dma_start(bhn_sb[:, :], bhn_d[:, :])
            for m in range(MT):
                nc.sync.dma_start(bi_sb[:, m : m + 1], bi_d[m : m + 1, :])
            for kk in range(KT):
                r0, r1 = kk * 128, (kk + 1) * 128
                nc.sync.dma_start(xT_sb[:, kk * T : (kk + 1) * T], xT_d[r0:r1, :])
                nc.sync.dma_start(wi_sb[:, kk * 768 : (kk + 1) * 768], wi_d[r0:r1, :])
                nc.sync.dma_start(wh_sb[:, kk * 768 : (kk + 1) * 768], wh_d[r0:r1, :])
                nc.sync.dma_start(H[:, kk * B0 : kk * B0 + B0], h0_d[r0:r1, :])

            # ---- input projections: gx = W_i^T @ x^T (+ b_i), bf16 out ----
            for n in range(2):
                c0 = n * 512
                for m in range(MT):
                    ps = pp.tile([128, 512], f32, tag=f"ps{m}")
                    for kk in range(KT):
                        nc.tensor.matmul(
                            ps[:, :],
                            lhsT=wi_sb[:, kk * 768 + m * 128 : kk * 768 + (m + 1) * 128],
                            rhs=xT_sb[:, kk * T + c0 : kk * T + c0 + 512],
                            start=(kk == 0),
                            stop=(kk == KT - 1),
                        )
                    nc.scalar.activation(
                        gx_sb[:, m * T + c0 : m * T + c0 + 512],
                        ps[:, :],
                        mybir.ActivationFunctionType.Identity,
                        bias=bi_sb[:, m : m + 1],
                    )

            # ---- recurrent rounds ----
            import concourse.bass as bass_mod

            off = 0
            hb_prev = None  # own-slice h from previous round: [2][128, cols]
            for k in range(R):
                B = Bk[k]
                Bn = Bk[k + 1] if k + 1 < R else 0
                hb = [wk.tile([128, B], bf16, tag=f"hb{t}", name=f"hb{t}_{k}") for t in range(2)]
                co = 0
                for cw in _chunks(B):
                    pss = []
                    for m in range(MT):
                        ps = pp.tile([128, cw], f32, tag=f"ps{m}")
                        pss.append(ps)
                        if m < 4:
                            nc.tensor.matmul(
                                ps[:, :],
                                lhsT=eye_sb[:, :],
                                rhs=gx_sb[:, m * 1024 + off + co : m * 1024 + off + co + cw],
                                start=True,
                                stop=False,
                            )
                        else:
                            nc.tensor.matmul(
                                ps[:, :],
                                lhsT=bhn_sb[0:1, (m - 4) * 128 : (m - 3) * 128],
                                rhs=ones_sb[0:1, co : co + cw],
                                start=True,
                                stop=False,
                            )
                        for kk in range(KT):
                            nc.tensor.matmul(
                                ps[:, :],
                                lhsT=wh_sb[:, kk * 768 + m * 128 : kk * 768 + (m + 1) * 128],
                                rhs=H[:, kk * B0 + co : kk * B0 + co + cw],
                                start=False,
                                stop=(kk == KT - 1),
                            )
                    for t in range(2):
                        r_ = wk.tile([128, cw], f32, tag=f"r{t}")
                        z_ = wk.tile([128, cw], f32, tag=f"z{t}")
                        m1 = wk.tile([128, cw], f32, tag=f"m1{t}")
                        m2 = wk.tile([128, cw], f32, tag=f"m2{t}")
                        n_ = wk.tile([128, cw], f32, tag=f"n{t}")
                        d_ = wk.tile([128, cw], f32, tag=f"d{t}")
                        e_ = wk.tile([128, cw], f32, tag=f"e{t}")
                        nc.scalar.activation(
                            r_[:, :], pss[t][:, :], mybir.ActivationFunctionType.Sigmoid
                        )
                        nc.scalar.activation(
                            z_[:, :], pss[2 + t][:, :], mybir.ActivationFunctionType.Sigmoid
                        )
                        nc.vector.tensor_tensor(
                            m1[:, :], r_[:, :], pss[4 + t][:, :], mybir.AluOpType.mult
                        )
                        nc.vector.tensor_tensor(
                            m2[:, :],
                            m1[:, :],
                            gx_sb[:, (4 + t) * 1024 + off + co : (4 + t) * 1024 + off + co + cw],
                            mybir.AluOpType.add,
                        )
                        nc.scalar.activation(
                            n_[:, :], m2[:, :], mybir.ActivationFunctionType.Tanh
                        )
                        # h_prev own slice
                        if k == 0:
                            hprev_ap = None
                        else:
                            hprev_ap = hb_prev[t][:, co : co + cw]
                        if hprev_ap is None:
                            # d = h0own - n
                            h0o = wk.tile([128, cw], bf16, tag=f"h0o{t}")
                            nc.sync.dma_start(
                                h0o[:, :], h0own_d[t * 128 : (t + 1) * 128, co : co + cw]
                            )
                            nc.vector.tensor_tensor(
                                d_[:, :], h0o[:, :], n_[:, :], mybir.AluOpType.subtract
                            )
                        else:
                            nc.vector.tensor_tensor(
                                d_[:, :], hprev_ap, n_[:, :], mybir.AluOpType.subtract
                            )
                        nc.vector.tensor_tensor(
                            e_[:, :], z_[:, :], d_[:, :], mybir.AluOpType.mult
                        )
                        nc.vector.tensor_tensor(
                            hb[t][:, co : co + cw], n_[:, :], e_[:, :], mybir.AluOpType.add
                        )
                        nc.sync.dma_start(
                            y_d[t * 128 : (t + 1) * 128, off + co : off + co + cw],
                            hb[t][:, co : co + cw],
                        )
                    co += cw
                if Bn > 0:
                    cc_in = nc.dram_tensor(f"ccin{k}", [2 * 128, Bn], bf16)
                    cc_out = nc.dram_tensor(
                        f"ccout{k}", [D, Bn], bf16, addr_space="Shared"
                    )
                    for t in range(2):
                        nc.sync.dma_start(
                            cc_in[t * 128 : (t + 1) * 128, :], hb[t][:, :Bn]
                        )
                    nc.gpsimd.collective_compute(
                        "AllGather",
                        mybir.AluOpType.bypass,
                        replica_groups=[list(range(NC))],
                        ins=[cc_in[:, :].opt()],
                        outs=[cc_out[:, :].opt()],
                    )
                    for kk in range(KT):
                        nc.sync.dma_start(
                            H[:, kk * B0 : kk * B0 + Bn],
                            cc_out[kk * 128 : (kk + 1) * 128, :],
                        )
                hb_prev = hb
                off += B
    nc.compile()
    return nc


def kernel(**inputs):
    x = np.asarray(inputs["inputs"], np.float32)
    term = np.asarray(inputs["terminations"]).astype(bool)
    last_state = np.asarray(inputs["last_state"], np.float32)
    w_i = [np.asarray(inputs[k], np.float32) for k in ("w_ir", "w_iz", "w_in")]
    b_i = [np.asarray(inputs[k], np.float32) for k in ("b_ir", "b_iz", "b_in")]
    w_h = [np.asarray(inputs[k], np.float32) for k in ("w_hr", "w_hz", "w_hn")]
    b_hn = np.asarray(inputs["b_hn"], np.float32)
    T = x.shape[0]

    p, l, Bk, rows = _segments(term)
    B0 = Bk[0]

    xT = np.ascontiguousarray(x[rows].T).astype(BF)  # [D, T] permuted
    h0 = np.zeros((D, B0), np.float32)
    for j in range(B0):
        if p[j] == 0 and not term[0]:
            h0[:, j] = last_state
    h0 = h0.astype(BF)
    eye = np.eye(128, dtype=np.float32).astype(BF)

    in_maps = []
    for c in range(NC):
        cs = slice(c * SLICE, (c + 1) * SLICE)
        wi_c = np.concatenate([w[:, cs] for w in w_i], axis=1).astype(BF)
        wh_c = np.concatenate([w[:, cs] for w in w_h], axis=1).astype(BF)
        bi_c = np.concatenate([b[cs] for b in b_i]).astype(np.float32).reshape(MT, 128)
        bhn_c = b_hn[cs].astype(BF).reshape(1, SLICE)
        in_maps.append(
            {
                "xT": xT,
                "wi": wi_c,
                "wh": wh_c,
                "bi": bi_c,
                "bhn": bhn_c,
                "h0": h0,
                "h0own": np.ascontiguousarray(h0[cs, :]),
                "eye": eye,
            }
        )

    trace = _maybe_enable_trace()
    nc = _build(Bk)
    from concourse.bass_utils import run_bass_kernel_spmd

    res = run_bass_kernel_spmd(
        nc, in_maps, core_ids=list(range(NC)), trace=trace
    )
    if res.exec_time_ns is not None:
        print(f"HW exec time: {res.exec_time_ns} ns")

    yT = np.concatenate(
        [res.results[c]["y"].astype(np.float32) for c in range(NC)], axis=0
    )  # [D, T] permuted
    y = np.empty((T, D), np.float32)
    y[rows] = yT.T
    return (y, y.copy())
